# revision 27
# baseline (speedup 1.0000x reference)
"""Multi-head attention (S=2048, B=2, D=1024, H=16) on 8 Trainium2 NeuronCores.

Sharding: tensor-parallel over heads. Each core computes 2 heads end-to-end
(QKV projections restricted to its 128 output dims, attention, and the
row-parallel slice of the output projection). The host sums the 8 partial
outputs (row-parallel Wo ==> partial sums) and adds bo.

All compute is fp16 with fp32 PSUM accumulation (fp8 was measured to cost
~8% output error: attention concentrates on few keys, so quantization noise
does not average out). The softmax denominator comes free from a ones-column
appended to V.

Design notes from profiling:
- Full-row (128-contraction) matmuls hide LDWEIGHTS in the background
  weight buffer (216ns steady at N=512); tile_position'd 64-row matmuls do
  not. So per-head K is stored zero-padded to 128 rows and score matmuls
  are plain full-row ones.
- The PE FIFO means any instruction waiting on a semaphore blocks all later
  PE work. The pipeline is software-pipelined one q-chunk deep: during the
  score/exp phase of chunk i, the PE interleaves the PV accumulation of
  chunk i-1 (whose exp tiles are all ready), plus projection/output fillers.
- Input DMAs are 1MB column-quarters of one big [128, KT, S] tile per
  tensor (small DMAs pay a ~1us fixed cost each and starve the startup).
"""

import math

import numpy as np

S, B, D, H = 2048, 2, 1024, 16
DK = D // H               # 64
NCORES = 8
HLOC = H // NCORES        # heads per core = 2
DLOC = HLOC * DK          # local output dims per core = 128
T = S * B                 # tokens = 4096
KT = D // 128             # contraction tiles = 8
NQC = S // 512            # query chunks per batch = 4
NKB = S // 128            # key blocks = 16
NTT = S // 128            # token tiles per batch = 16
SCALE = 1.0 / math.sqrt(DK)

_prog_cache = {}


def _build(masked: bool):
    import concourse.mybir as mybir
    import concourse.tile as tile
    from concourse import bacc

    f16 = mybir.dt.float16
    f32 = mybir.dt.float32
    EXP = mybir.ActivationFunctionType.Exp
    MUL = mybir.AluOpType.mult
    ADD = mybir.AluOpType.add

    nc = bacc.Bacc("TRN2", target_bir_lowering=False, debug=False)

    def din(name, shape, dt=f16):
        return nc.dram_tensor(name, shape, dt, kind="ExternalInput").ap()

    xq = din("xq", [D, B, S])          # query^T
    xk = din("xk", [D, B, S])          # key^T
    xv = din("xv", [D, B, S])          # value^T
    wq = din("wq", [128, KT * DLOC])   # [p, kt, m] = W[hs+m, kt*128+p]
    wk = din("wk", [128, KT * DLOC])
    wv = din("wv", [128, KT * DLOC])
    wo = din("wo", [DLOC, D])          # Wo[:, hs:hs+128].T
    bq = din("bq", [DLOC], f32)
    bk = din("bk", [DLOC], f32)
    bv = din("bv", [DLOC], f32)
    mb = din("mb", [S], f32)           # additive mask bias per key (0 / -1e30)
    out = nc.dram_tensor("out", [S, B, D], f16, kind="ExternalOutput").ap()

    with tile.TileContext(nc) as tc:
        with (
            tc.tile_pool(name="wsb", bufs=1) as wsb,
            tc.tile_pool(name="xsb", bufs=1) as xsb,
            tc.tile_pool(name="qkv", bufs=1) as qkv,
            tc.tile_pool(name="esb", bufs=18) as esb,
            tc.tile_pool(name="nrm", bufs=2) as nrm,
            tc.tile_pool(name="osb", bufs=4) as osb,
            tc.tile_pool(name="pj", bufs=2, space="PSUM") as pj,
            tc.tile_pool(name="psc", bufs=2, space="PSUM") as psc,
            tc.tile_pool(name="pcx", bufs=1, space="PSUM") as pcx,
        ):
            # ---- constants / weights -------------------------------------
            # weights/biases ride the gpsimd queue so the sync HWDGE ring is
            # free for xk0 from t=0 (ring credits serialize w before xk0
            # otherwise)
            w_sb = {}
            for name, ap in (("wq", wq), ("wk", wk), ("wv", wv)):
                t = wsb.tile([128, KT, DLOC], f16, tag=name)
                nc.gpsimd.dma_start(out=t, in_=ap.rearrange("p (kt m) -> p kt m", kt=KT))
                w_sb[name] = t
            wo_sb = wsb.tile([DLOC, D], f16, tag="wo")
            nc.gpsimd.dma_start(out=wo_sb, in_=wo)
            bq_sb = wsb.tile([DLOC, 1], f32, tag="bq")
            nc.gpsimd.dma_start(out=bq_sb, in_=bq.unsqueeze(1))
            bk_sb = wsb.tile([DLOC, 1], f32, tag="bk")
            nc.gpsimd.dma_start(out=bk_sb, in_=bk.unsqueeze(1))
            bv_row = wsb.tile([1, DLOC], f32, tag="bv_row")
            nc.gpsimd.dma_start(out=bv_row, in_=bv.unsqueeze(0))
            bv_bc = wsb.tile([128, DLOC], f32, tag="bv_bc")
            nc.gpsimd.partition_broadcast(bv_bc, bv_row)
            mb_sb = wsb.tile([128, NKB], f32, tag="mb")
            nc.gpsimd.dma_start(out=mb_sb, in_=mb.rearrange("(kb p) -> p kb", p=128))
            # preload the exp table set while input DMAs are in flight
            warm_e = wsb.tile([1, 1], f16, tag="warm_e")
            nc.scalar.activation(warm_e, mb_sb[0:1, 0:1], EXP, scale=1.0)

            # persistent per-batch activations
            qT = [qkv.tile([DLOC, S], f16, tag=f"qT{b}", name=f"qT{b}") for b in range(B)]
            # per-head K, zero-padded to 128 rows: head h occupies rows
            # h*64:(h+1)*64, the rest stays 0 -> score matmuls are full-row
            # (LDWEIGHTS hides in the background weight buffer).
            kTz = [[qkv.tile([128, S], f16, tag=f"kT{b}{h}", name=f"kT{b}{h}")
                    for h in range(HLOC)] for b in range(B)]
            # V per (key-block, head): [keys=128, 68] with ones in col 64
            vv = [qkv.tile([128, NKB, HLOC, 68], f16, tag=f"vv{b}", name=f"vv{b}")
                  for b in range(B)]
            for b in range(B):
                for h in range(HLOC):
                    nc.vector.memset(kTz[b][h], 0.0)
                nc.vector.memset(vv[b], 0.0)
                nc.vector.memset(vv[b][:, :, :, 64:65], 1.0)
            ctxn = [qkv.tile([DLOC, S], f16, tag=f"ctxn{b}", name=f"ctxn{b}") for b in range(B)]

            def load_x(ap, name, b, eng):
                """One big [128, KT, S] tile, four 1MB column-quarter DMAs
                (all kt per quarter, so proj chunk c unblocks at quarter c).
                Returns per-kt view APs."""
                big = xsb.tile([128, KT, S], f16, tag=f"x{name}", name=f"x{name}{b}")
                src = ap[:, b, :].rearrange("(kt p) s -> p kt s", p=128)
                for i in range(4):
                    eng.dma_start(out=big[:, :, i * 512:(i + 1) * 512],
                                  in_=src[:, :, i * 512:(i + 1) * 512])
                return [big[:, kt, :] for kt in range(KT)]

            def proj_q_chunk(b, xt, qc):
                ps = pj.tile([DLOC, 512], f32, tag="pj", name="ps")
                sl = slice(qc * 512, (qc + 1) * 512)
                for kt in range(KT):
                    nc.tensor.matmul(ps, w_sb["wq"][:, kt, :], xt[kt][:, sl],
                                     start=(kt == 0), stop=(kt == KT - 1))
                nc.vector.tensor_scalar(out=qT[b][:, sl], in0=ps,
                                        scalar1=bq_sb, scalar2=None, op0=ADD)

            def proj_k_chunk(b, xt, qc):
                ps = pj.tile([DLOC, 512], f32, tag="pj", name="ps")
                sl = slice(qc * 512, (qc + 1) * 512)
                for kt in range(KT):
                    nc.tensor.matmul(ps, w_sb["wk"][:, kt, :], xt[kt][:, sl],
                                     start=(kt == 0), stop=(kt == KT - 1))
                # split the two heads into their zero-padded K tiles
                for h in range(HLOC):
                    hsl = slice(h * 64, (h + 1) * 64)
                    nc.vector.tensor_scalar(out=kTz[b][h][hsl, sl],
                                            in0=ps[hsl, :],
                                            scalar1=bk_sb[hsl, :], scalar2=None,
                                            op0=ADD)

            def proj_v_tt(b, xt, tts):
                for tt in tts:
                    ps = pj.tile([128, DLOC], f32, tag="pj", name="ps")
                    sl = slice(tt * 128, (tt + 1) * 128)
                    for kt in range(KT):
                        nc.tensor.matmul(ps, xt[kt][:, sl], w_sb["wv"][:, kt, :],
                                         start=(kt == 0), stop=(kt == KT - 1))
                    for h in range(HLOC):
                        nc.vector.tensor_tensor(
                            out=vv[b][:, tt, h, 0:64],
                            in0=ps[:, h * 64:(h + 1) * 64],
                            in1=bv_bc[:, h * 64:(h + 1) * 64], op=ADD)

            def outproj_tt(b, tts, on_scalar=False):
                for tt in tts:
                    tsl = slice(tt * 128, (tt + 1) * 128)
                    for eh in range(2):
                        po = pj.tile([128, 512], f32, tag="pj", name="po")
                        nc.tensor.matmul(po, ctxn[b][:, tsl],
                                         wo_sb[:, eh * 512:(eh + 1) * 512],
                                         start=True, stop=True)
                        oc = osb.tile([128, 512], f16, tag="oc", name="oc")
                        if on_scalar:
                            nc.scalar.copy(oc, po)
                        else:
                            nc.vector.tensor_copy(oc, po)
                        nc.gpsimd.dma_start(
                            out=out[tsl, b, eh * 512:(eh + 1) * 512], in_=oc)

            # ---- software-pipelined attention ----------------------------
            # carry = (b, qc, ets, pctx) of the previous q-chunk whose PV
            # accumulation runs interleaved into the current s-phase.
            def pv_pair(carry, kb, last):
                cb, cqc, ets, pctx = carry
                et = ets.pop(kb)
                for h in range(HLOC):
                    nc.tensor.matmul(pctx[h], vv[cb][:, kb, h, 0:65],
                                     et[:, h, :],
                                     start=(kb == 0), stop=(kb == NKB - 1))

            def cd_finish(carry):
                cb, cqc, ets, pctx = carry
                qsl = slice(cqc * 512, (cqc + 1) * 512)
                for h in range(HLOC):
                    hsl = slice(h * 64, (h + 1) * 64)
                    cl = nrm.tile([1, 512], f32, tag="cl", name="cl")
                    nc.vector.tensor_copy(cl, pctx[h][64:65, :])
                    rl = nrm.tile([1, 512], f32, tag="rl", name="rl")
                    nc.vector.reciprocal_approx_fast(rl, cl)
                    rl_bc = nrm.tile([64, 512], f32, tag="rlb", name="rlb")
                    nc.gpsimd.partition_broadcast(rl_bc, rl)
                    nc.vector.tensor_tensor(out=ctxn[cb][hsl, qsl],
                                            in0=pctx[h][0:64, :], in1=rl_bc,
                                            op=MUL)

            def s_phase(b, qc, carry, inj):
                """16 score+exp steps; interleaves carry's PV pairs and the
                injected fillers. Returns this chunk's carry."""
                qsl = slice(qc * 512, (qc + 1) * 512)
                pctx = [pcx.tile([65, 512], f32, tag=f"cx{h}", name=f"cx{h}")
                        for h in range(HLOC)]
                ets = {}
                LAG = 2   # carry-PV trails by 2 steps so same-step injected
                # producers (e.g. vproj feeding vv) are emitted in time
                for kb in range(NKB):
                    t = psc.tile([128, HLOC, 512], f32, tag="sc", name="sc")
                    ksl = slice(kb * 128, (kb + 1) * 128)
                    for h in range(HLOC):
                        nc.tensor.matmul(t[:, h, :], kTz[b][h][:, ksl],
                                         qT[b][:, qsl], start=True, stop=True)
                    et = esb.tile([128, HLOC, 512], f16, tag="e", name="et")
                    if masked:
                        nc.scalar.activation(et, t, EXP,
                                             bias=mb_sb[:, kb:kb + 1], scale=SCALE)
                    else:
                        nc.scalar.activation(et, t, EXP, scale=SCALE)
                    ets[kb] = et
                    if carry is not None and kb >= LAG:
                        pv_pair(carry, kb - LAG, False)
                    th = inj.get(kb)
                    if th is not None:
                        th()
                if carry is not None:
                    for kb in range(NKB - LAG, NKB):
                        pv_pair(carry, kb, kb == NKB - 1)
                    cd_finish(carry)
                return (b, qc, ets, pctx)

            # ---- schedule ------------------------------------------------
            # DMA queues: sync = xk0, xk1, xq1; scalar = xq0 then xv0 (the
            # FIFO delays xv0 until xq0 is fully landed, and xv0's four
            # issue slots are the only scalar DMAs inside the exp stream);
            # gpsimd = xv1 (its tile-ring WAR delays it behind vproj b0)
            # and the output stores.
            xk_t = load_x(xk, "k", 0, nc.sync)
            xq_t = load_x(xq, "q", 0, nc.scalar)
            for wu in range(40):
                jp = pj.tile([128, 512], f32, tag="pj", name="jp")
                nc.tensor.matmul(jp, w_sb["wq"][:, wu % 8, :],
                                 w_sb["wk"][:, (wu % 2) * 4:(wu % 2) * 4 + 4, :],
                                 start=True, stop=True)
            proj_k_chunk(0, xk_t, 0)
            proj_q_chunk(0, xq_t, 0)
            # gate xv0's transfers behind the first q-proj so it can't steal
            # HBM bandwidth from the xk0/xq0 ramp: a 1-element copy into the
            # xv tile makes the DMA's WAR wait on it
            xv_big = xsb.tile([128, KT, S], f16, tag="xv", name="xv0")
            nc.vector.tensor_copy(xv_big[0:1, 0:1, 0:1], qT[0][0:1, 0:1])
            src = xv[:, 0, :].rearrange("(kt p) s -> p kt s", p=128)
            for i in range(4):
                nc.gpsimd.dma_start(out=xv_big[:, :, i * 512:(i + 1) * 512],
                                    in_=src[:, :, i * 512:(i + 1) * 512])
            xv_t = [xv_big[:, kt, :] for kt in range(KT)]
            x2 = {"v": xv_t}

            # Ring hazard rule: a tensor's generation-2 load must be emitted
            # AFTER the last emitted reader of generation 1 (bufs=1 rings).
            c = s_phase(0, 0, None, {
                1: lambda: proj_k_chunk(0, xk_t, 1),
                4: lambda: proj_k_chunk(0, xk_t, 2),
                7: lambda: proj_k_chunk(0, xk_t, 3),
                8: lambda: x2.update(k=load_x(xk, "k", 1, nc.sync)),
                9: lambda: proj_q_chunk(0, xq_t, 1),
            })
            c = s_phase(0, 1, c, {
                1: lambda: proj_v_tt(0, x2["v"], [0, 1]),
                2: lambda: proj_v_tt(0, x2["v"], [2, 3]),
                3: lambda: proj_v_tt(0, x2["v"], [4, 5]),
                4: lambda: proj_v_tt(0, x2["v"], [6, 7]),
                5: lambda: proj_v_tt(0, x2["v"], [8, 9]),
                6: lambda: proj_v_tt(0, x2["v"], [10, 11]),
                7: lambda: proj_v_tt(0, x2["v"], [12, 13]),
                8: lambda: proj_v_tt(0, x2["v"], [14, 15]),
                10: lambda: x2.update(v2=load_x(xv, "v", 1, nc.gpsimd)),
                14: lambda: proj_q_chunk(0, xq_t, 2),
            })
            c = s_phase(0, 2, c, {
                1: lambda: proj_q_chunk(0, xq_t, 3),
                2: lambda: x2.update(q=load_x(xq, "q", 1, nc.sync)),
                3: lambda: proj_k_chunk(1, x2["k"], 0),
                6: lambda: proj_k_chunk(1, x2["k"], 1),
                9: lambda: proj_k_chunk(1, x2["k"], 2),
                12: lambda: proj_k_chunk(1, x2["k"], 3),
            })
            c = s_phase(0, 3, c, {
                1: lambda: proj_q_chunk(1, x2["q"], 0),
                4: lambda: proj_q_chunk(1, x2["q"], 1),
                7: lambda: outproj_tt(0, [0]),
                8: lambda: outproj_tt(0, [1]),
                9: lambda: outproj_tt(0, [2]),
                10: lambda: outproj_tt(0, [3]),
                11: lambda: outproj_tt(0, [4]),
                12: lambda: outproj_tt(0, [5]),
                13: lambda: proj_v_tt(1, x2["v2"], [0, 1]),
                14: lambda: proj_v_tt(1, x2["v2"], [2, 3]),
            })
            c = s_phase(1, 0, c, {
                1: lambda: proj_v_tt(1, x2["v2"], [4, 5]),
                3: lambda: proj_v_tt(1, x2["v2"], [6, 7]),
                5: lambda: proj_v_tt(1, x2["v2"], [8, 9]),
                7: lambda: proj_v_tt(1, x2["v2"], [10, 11]),
                9: lambda: proj_v_tt(1, x2["v2"], [12, 13]),
                11: lambda: proj_v_tt(1, x2["v2"], [14, 15]),
                13: lambda: outproj_tt(0, [6]),
                14: lambda: outproj_tt(0, [7]),
            })
            c = s_phase(1, 1, c, {
                1: lambda: outproj_tt(0, [8]),
                3: lambda: outproj_tt(0, [9]),
                5: lambda: outproj_tt(0, [10]),
                7: lambda: outproj_tt(0, [11]),
                9: lambda: outproj_tt(0, [12]),
                11: lambda: outproj_tt(0, [13]),
                12: lambda: proj_q_chunk(1, x2["q"], 2),
                13: lambda: outproj_tt(0, [14]),
                14: lambda: outproj_tt(0, [15]),
            })
            c = s_phase(1, 2, c, {
                1: lambda: outproj_tt(1, [0]),
                4: lambda: outproj_tt(1, [1]),
                7: lambda: outproj_tt(1, [2]),
                10: lambda: outproj_tt(1, [3]),
                12: lambda: proj_q_chunk(1, x2["q"], 3),
            })
            c = s_phase(1, 3, c, {
                1: lambda: outproj_tt(1, [4]),
                4: lambda: outproj_tt(1, [5]),
                7: lambda: outproj_tt(1, [6]),
                10: lambda: outproj_tt(1, [7]),
            })
            # drain: PV of the last chunk interleaved with ready outprojs
            # (oc copies alternate scalar/vector - ACT is free by now)
            for kb in range(NKB):
                pv_pair(c, kb, kb == NKB - 1)
                if kb in (2, 6, 10, 14):
                    outproj_tt(1, [8 + kb // 4], on_scalar=(kb % 8 == 2))
            cd_finish(c)
            outproj_tt(1, [12], on_scalar=True)
            outproj_tt(1, [13])
            outproj_tt(1, [14], on_scalar=True)
            outproj_tt(1, [15])

    nc.compile()
    return nc


def _get_prog(masked: bool):
    key = masked
    if key not in _prog_cache:
        _prog_cache[key] = _build(masked)
    return _prog_cache[key]


def kernel(query, key, value, mask, Wq, bq, Wk, bk, Wv, bv, Wo, bo):
    from concourse.bass_utils import run_bass_kernel_spmd

    query = np.asarray(query)
    key = np.asarray(key)
    value = np.asarray(value)
    mask = np.asarray(mask)
    Wq, bq = np.asarray(Wq), np.asarray(bq)
    Wk, bk = np.asarray(Wk), np.asarray(bk)
    Wv, bv = np.asarray(Wv), np.asarray(bv)
    Wo, bo = np.asarray(Wo), np.asarray(bo)

    masked = not bool(mask.all())
    nc = _get_prog(masked)

    def t16(x):  # [S, B, D] -> contiguous [D, B, S] fp16
        return np.ascontiguousarray(x.transpose(2, 1, 0).astype(np.float16))

    def warr(W, hs):  # [128, KT*128]: row p = concat_kt W[hs+m, kt*128+p]
        wt = W[hs:hs + DLOC, :].T.astype(np.float16)       # [kt*128+p, m]
        return np.ascontiguousarray(
            wt.reshape(KT, 128, DLOC).transpose(1, 0, 2).reshape(128, KT * DLOC))

    xq, xk, xv = t16(query), t16(key), t16(value)
    mb = np.where(mask.reshape(S), 0.0, -1e30).astype(np.float32)

    in_maps = []
    for c in range(NCORES):
        hs = c * DLOC
        in_maps.append({
            "xq": xq, "xk": xk, "xv": xv,
            "wq": warr(Wq, hs),
            "wk": warr(Wk, hs),
            "wv": warr(Wv, hs),
            "wo": np.ascontiguousarray(Wo[:, hs:hs + DLOC].T.astype(np.float16)),
            "bq": bq[hs:hs + DLOC].astype(np.float32),
            "bk": bk[hs:hs + DLOC].astype(np.float32),
            "bv": bv[hs:hs + DLOC].astype(np.float32),
            "mb": mb,
        })

    res = run_bass_kernel_spmd(nc, in_maps, core_ids=list(range(NCORES)))
    acc = res.results[0]["out"].astype(np.float64)
    for c in range(1, NCORES):
        acc += res.results[c]["out"]
    acc += bo.astype(np.float64)
    return acc.astype(np.float32)


# revision 29
# speedup vs baseline: 1.0377x; 1.0377x over previous
"""Multi-head attention (S=2048, B=2, D=1024, H=16) on 8 Trainium2 NeuronCores.

Sharding: tensor-parallel over heads. Each core computes 2 heads end-to-end
(QKV projections restricted to its 128 output dims, attention, and the
row-parallel slice of the output projection). The host sums the 8 partial
outputs (row-parallel Wo ==> partial sums) and adds bo.

All compute is fp16 with fp32 PSUM accumulation (fp8 was measured to cost
~8% output error: attention concentrates on few keys, so quantization noise
does not average out). The softmax denominator comes free from a ones-column
appended to V.

Design notes from profiling:
- Full-row (128-contraction) matmuls hide LDWEIGHTS in the background
  weight buffer (216ns steady at N=512); tile_position'd 64-row matmuls do
  not. So per-head K is stored zero-padded to 128 rows and score matmuls
  are plain full-row ones.
- The PE FIFO means any instruction waiting on a semaphore blocks all later
  PE work. The pipeline is software-pipelined one q-chunk deep: during the
  score/exp phase of chunk i, the PE interleaves the PV accumulation of
  chunk i-1 (whose exp tiles are all ready), plus projection/output fillers.
- Input DMAs are 1MB column-quarters of one big [128, KT, S] tile per
  tensor (small DMAs pay a ~1us fixed cost each and starve the startup).
"""

import math

import numpy as np

S, B, D, H = 2048, 2, 1024, 16
DK = D // H               # 64
NCORES = 8
HLOC = H // NCORES        # heads per core = 2
DLOC = HLOC * DK          # local output dims per core = 128
T = S * B                 # tokens = 4096
KT = D // 128             # contraction tiles = 8
NQC = S // 512            # query chunks per batch = 4
NKB = S // 128            # key blocks = 16
NTT = S // 128            # token tiles per batch = 16
SCALE = 1.0 / math.sqrt(DK)

_prog_cache = {}


def _build(masked: bool):
    import concourse.mybir as mybir
    import concourse.tile as tile
    from concourse import bacc

    f16 = mybir.dt.float16
    f32 = mybir.dt.float32
    EXP = mybir.ActivationFunctionType.Exp
    MUL = mybir.AluOpType.mult
    ADD = mybir.AluOpType.add

    nc = bacc.Bacc("TRN2", target_bir_lowering=False, debug=False)

    def din(name, shape, dt=f16):
        return nc.dram_tensor(name, shape, dt, kind="ExternalInput").ap()

    xq = din("xq", [D, B, S])          # query^T
    xk = din("xk", [D, B, S])          # key^T
    xv = din("xv", [D, B, S])          # value^T
    wq = din("wq", [128, KT * DLOC])   # [p, kt, m] = W[hs+m, kt*128+p]
    wk = din("wk", [128, KT * DLOC])
    wv = din("wv", [128, KT * DLOC])
    wo = din("wo", [DLOC, D])          # Wo[:, hs:hs+128].T
    bq = din("bq", [DLOC], f32)
    bk = din("bk", [DLOC], f32)
    bv = din("bv", [DLOC], f32)
    mb = din("mb", [S], f32)           # additive mask bias per key (0 / -1e30)
    out = nc.dram_tensor("out", [S, B, D], f16, kind="ExternalOutput").ap()

    with tile.TileContext(nc) as tc:
        with (
            tc.tile_pool(name="wsb", bufs=1) as wsb,
            tc.tile_pool(name="xsb", bufs=1) as xsb,
            tc.tile_pool(name="qkv", bufs=1) as qkv,
            tc.tile_pool(name="esb", bufs=18) as esb,
            tc.tile_pool(name="nrm", bufs=2) as nrm,
            tc.tile_pool(name="osb", bufs=4) as osb,
            tc.tile_pool(name="pj", bufs=2, space="PSUM") as pj,
            tc.tile_pool(name="psc", bufs=2, space="PSUM") as psc,
            tc.tile_pool(name="pcx", bufs=1, space="PSUM") as pcx,
        ):
            # ---- constants / weights -------------------------------------
            w_sb = {}
            for name, ap in (("wq", wq), ("wk", wk), ("wv", wv)):
                t = wsb.tile([128, KT, DLOC], f16, tag=name)
                nc.sync.dma_start(out=t, in_=ap.rearrange("p (kt m) -> p kt m", kt=KT))
                w_sb[name] = t
            wo_sb = wsb.tile([DLOC, D], f16, tag="wo")
            nc.sync.dma_start(out=wo_sb, in_=wo)
            bq_sb = wsb.tile([DLOC, 1], f32, tag="bq")
            nc.sync.dma_start(out=bq_sb, in_=bq.unsqueeze(1))
            bk_sb = wsb.tile([DLOC, 1], f32, tag="bk")
            nc.sync.dma_start(out=bk_sb, in_=bk.unsqueeze(1))
            bv_row = wsb.tile([1, DLOC], f32, tag="bv_row")
            nc.sync.dma_start(out=bv_row, in_=bv.unsqueeze(0))
            bv_bc = wsb.tile([128, DLOC], f32, tag="bv_bc")
            nc.gpsimd.partition_broadcast(bv_bc, bv_row)
            mb_sb = wsb.tile([128, NKB], f32, tag="mb")
            nc.sync.dma_start(out=mb_sb, in_=mb.rearrange("(kb p) -> p kb", p=128))
            # preload the exp table set while input DMAs are in flight
            warm_e = wsb.tile([1, 1], f16, tag="warm_e")
            nc.scalar.activation(warm_e, mb_sb[0:1, 0:1], EXP, scale=1.0)

            # persistent per-batch activations
            qT = [qkv.tile([DLOC, S], f16, tag=f"qT{b}", name=f"qT{b}") for b in range(B)]
            # per-head K, zero-padded to 128 rows: head h occupies rows
            # h*64:(h+1)*64, the rest stays 0 -> score matmuls are full-row
            # (LDWEIGHTS hides in the background weight buffer).
            kTz = [[qkv.tile([128, S], f16, tag=f"kT{b}{h}", name=f"kT{b}{h}")
                    for h in range(HLOC)] for b in range(B)]
            # V per (key-block, head): [keys=128, 68] with ones in col 64
            vv = [qkv.tile([128, NKB, HLOC, 68], f16, tag=f"vv{b}", name=f"vv{b}")
                  for b in range(B)]
            for b in range(B):
                for h in range(HLOC):
                    nc.vector.memset(kTz[b][h], 0.0)
                nc.vector.memset(vv[b], 0.0)
                nc.vector.memset(vv[b][:, :, :, 64:65], 1.0)
            ctxn = [qkv.tile([DLOC, S], f16, tag=f"ctxn{b}", name=f"ctxn{b}") for b in range(B)]

            def load_x(ap, name, b, eng):
                """One big [128, KT, S] tile, four 1MB column-quarter DMAs
                (all kt per quarter, so proj chunk c unblocks at quarter c).
                Returns per-kt view APs."""
                big = xsb.tile([128, KT, S], f16, tag=f"x{name}", name=f"x{name}{b}")
                src = ap[:, b, :].rearrange("(kt p) s -> p kt s", p=128)
                for i in range(4):
                    eng.dma_start(out=big[:, :, i * 512:(i + 1) * 512],
                                  in_=src[:, :, i * 512:(i + 1) * 512])
                return [big[:, kt, :] for kt in range(KT)]

            def proj_q_chunk(b, xt, qc):
                ps = pj.tile([DLOC, 512], f32, tag="pj", name="ps")
                sl = slice(qc * 512, (qc + 1) * 512)
                for kt in range(KT):
                    nc.tensor.matmul(ps, w_sb["wq"][:, kt, :], xt[kt][:, sl],
                                     start=(kt == 0), stop=(kt == KT - 1))
                nc.vector.tensor_scalar(out=qT[b][:, sl], in0=ps,
                                        scalar1=bq_sb, scalar2=None, op0=ADD)

            def proj_k_chunk(b, xt, qc):
                ps = pj.tile([DLOC, 512], f32, tag="pj", name="ps")
                sl = slice(qc * 512, (qc + 1) * 512)
                for kt in range(KT):
                    nc.tensor.matmul(ps, w_sb["wk"][:, kt, :], xt[kt][:, sl],
                                     start=(kt == 0), stop=(kt == KT - 1))
                # split the two heads into their zero-padded K tiles
                for h in range(HLOC):
                    hsl = slice(h * 64, (h + 1) * 64)
                    nc.vector.tensor_scalar(out=kTz[b][h][hsl, sl],
                                            in0=ps[hsl, :],
                                            scalar1=bk_sb[hsl, :], scalar2=None,
                                            op0=ADD)

            def proj_v_tt(b, xt, tts):
                for tt in tts:
                    ps = pj.tile([128, DLOC], f32, tag="pj", name="ps")
                    sl = slice(tt * 128, (tt + 1) * 128)
                    for kt in range(KT):
                        nc.tensor.matmul(ps, xt[kt][:, sl], w_sb["wv"][:, kt, :],
                                         start=(kt == 0), stop=(kt == KT - 1))
                    for h in range(HLOC):
                        nc.vector.tensor_tensor(
                            out=vv[b][:, tt, h, 0:64],
                            in0=ps[:, h * 64:(h + 1) * 64],
                            in1=bv_bc[:, h * 64:(h + 1) * 64], op=ADD)

            def outproj_tt(b, tts, on_scalar=False):
                for tt in tts:
                    tsl = slice(tt * 128, (tt + 1) * 128)
                    for eh in range(2):
                        po = pj.tile([128, 512], f32, tag="pj", name="po")
                        nc.tensor.matmul(po, ctxn[b][:, tsl],
                                         wo_sb[:, eh * 512:(eh + 1) * 512],
                                         start=True, stop=True)
                        oc = osb.tile([128, 512], f16, tag="oc", name="oc")
                        if on_scalar:
                            nc.scalar.copy(oc, po)
                        else:
                            nc.vector.tensor_copy(oc, po)
                        nc.gpsimd.dma_start(
                            out=out[tsl, b, eh * 512:(eh + 1) * 512], in_=oc)

            # ---- software-pipelined attention ----------------------------
            # carry = (b, qc, ets, pctx) of the previous q-chunk whose PV
            # accumulation runs interleaved into the current s-phase.
            def pv_pair(carry, kb, last):
                cb, cqc, ets, pctx = carry
                et = ets.pop(kb)
                for h in range(HLOC):
                    nc.tensor.matmul(pctx[h], vv[cb][:, kb, h, 0:65],
                                     et[:, h, :],
                                     start=(kb == 0), stop=(kb == NKB - 1))

            def cd_finish(carry):
                cb, cqc, ets, pctx = carry
                qsl = slice(cqc * 512, (cqc + 1) * 512)
                for h in range(HLOC):
                    hsl = slice(h * 64, (h + 1) * 64)
                    cl = nrm.tile([1, 512], f32, tag="cl", name="cl")
                    nc.vector.tensor_copy(cl, pctx[h][64:65, :])
                    rl = nrm.tile([1, 512], f32, tag="rl", name="rl")
                    nc.vector.reciprocal_approx_fast(rl, cl)
                    rl_bc = nrm.tile([64, 512], f32, tag="rlb", name="rlb")
                    nc.gpsimd.partition_broadcast(rl_bc, rl)
                    nc.vector.tensor_tensor(out=ctxn[cb][hsl, qsl],
                                            in0=pctx[h][0:64, :], in1=rl_bc,
                                            op=MUL)

            def s_phase(b, qc, carry, inj):
                """16 score+exp steps; interleaves carry's PV pairs and the
                injected fillers. Returns this chunk's carry."""
                qsl = slice(qc * 512, (qc + 1) * 512)
                pctx = [pcx.tile([65, 512], f32, tag=f"cx{h}", name=f"cx{h}")
                        for h in range(HLOC)]
                ets = {}
                LAG = 2   # carry-PV trails by 2 steps so same-step injected
                # producers (e.g. vproj feeding vv) are emitted in time
                for kb in range(NKB):
                    t = psc.tile([128, HLOC, 512], f32, tag="sc", name="sc")
                    ksl = slice(kb * 128, (kb + 1) * 128)
                    for h in range(HLOC):
                        nc.tensor.matmul(t[:, h, :], kTz[b][h][:, ksl],
                                         qT[b][:, qsl], start=True, stop=True)
                    et = esb.tile([128, HLOC, 512], f16, tag="e", name="et")
                    if masked:
                        nc.scalar.activation(et, t, EXP,
                                             bias=mb_sb[:, kb:kb + 1], scale=SCALE)
                    else:
                        nc.scalar.activation(et, t, EXP, scale=SCALE)
                    ets[kb] = et
                    if carry is not None and kb >= LAG:
                        pv_pair(carry, kb - LAG, False)
                    th = inj.get(kb)
                    if th is not None:
                        th()
                if carry is not None:
                    for kb in range(NKB - LAG, NKB):
                        pv_pair(carry, kb, kb == NKB - 1)
                    cd_finish(carry)
                return (b, qc, ets, pctx)

            # ---- schedule ------------------------------------------------
            # DMA queues: sync = xk0, xk1, xq1; scalar = xq0 then xv0 (the
            # FIFO delays xv0 until xq0 is fully landed, and xv0's four
            # issue slots are the only scalar DMAs inside the exp stream);
            # gpsimd = xv1 (its tile-ring WAR delays it behind vproj b0)
            # and the output stores.
            xk_t = load_x(xk, "k", 0, nc.sync)
            xq_t = load_x(xq, "q", 0, nc.scalar)
            xv_t = load_x(xv, "v", 0, nc.gpsimd)
            x2 = {"v": xv_t}
            for wu in range(40):
                jp = pj.tile([128, 512], f32, tag="pj", name="jp")
                nc.tensor.matmul(jp, w_sb["wq"][:, wu % 8, :],
                                 w_sb["wk"][:, (wu % 2) * 4:(wu % 2) * 4 + 4, :],
                                 start=True, stop=True)
            proj_k_chunk(0, xk_t, 0)
            proj_q_chunk(0, xq_t, 0)

            # Ring hazard rule: a tensor's generation-2 load must be emitted
            # AFTER the last emitted reader of generation 1 (bufs=1 rings).
            c = s_phase(0, 0, None, {
                1: lambda: proj_k_chunk(0, xk_t, 1),
                4: lambda: proj_k_chunk(0, xk_t, 2),
                7: lambda: proj_k_chunk(0, xk_t, 3),
                8: lambda: x2.update(k=load_x(xk, "k", 1, nc.sync)),
                9: lambda: proj_q_chunk(0, xq_t, 1),
            })
            c = s_phase(0, 1, c, {
                1: lambda: proj_v_tt(0, x2["v"], [0, 1]),
                2: lambda: proj_v_tt(0, x2["v"], [2, 3]),
                3: lambda: proj_v_tt(0, x2["v"], [4, 5]),
                4: lambda: proj_v_tt(0, x2["v"], [6, 7]),
                5: lambda: proj_v_tt(0, x2["v"], [8, 9]),
                6: lambda: proj_v_tt(0, x2["v"], [10, 11]),
                7: lambda: proj_v_tt(0, x2["v"], [12, 13]),
                8: lambda: proj_v_tt(0, x2["v"], [14, 15]),
                10: lambda: x2.update(v2=load_x(xv, "v", 1, nc.gpsimd)),
                14: lambda: proj_q_chunk(0, xq_t, 2),
            })
            c = s_phase(0, 2, c, {
                1: lambda: proj_q_chunk(0, xq_t, 3),
                2: lambda: x2.update(q=load_x(xq, "q", 1, nc.sync)),
                3: lambda: proj_k_chunk(1, x2["k"], 0),
                6: lambda: proj_k_chunk(1, x2["k"], 1),
                9: lambda: proj_k_chunk(1, x2["k"], 2),
                12: lambda: proj_k_chunk(1, x2["k"], 3),
            })
            c = s_phase(0, 3, c, {
                1: lambda: proj_q_chunk(1, x2["q"], 0),
                4: lambda: proj_q_chunk(1, x2["q"], 1),
                7: lambda: outproj_tt(0, [0]),
                8: lambda: outproj_tt(0, [1]),
                9: lambda: outproj_tt(0, [2]),
                10: lambda: outproj_tt(0, [3]),
                11: lambda: outproj_tt(0, [4]),
                12: lambda: outproj_tt(0, [5]),
                13: lambda: proj_v_tt(1, x2["v2"], [0, 1]),
                14: lambda: proj_v_tt(1, x2["v2"], [2, 3]),
            })
            c = s_phase(1, 0, c, {
                1: lambda: proj_v_tt(1, x2["v2"], [4, 5]),
                3: lambda: proj_v_tt(1, x2["v2"], [6, 7]),
                5: lambda: proj_v_tt(1, x2["v2"], [8, 9]),
                7: lambda: proj_v_tt(1, x2["v2"], [10, 11]),
                9: lambda: proj_v_tt(1, x2["v2"], [12, 13]),
                11: lambda: proj_v_tt(1, x2["v2"], [14, 15]),
                13: lambda: outproj_tt(0, [6]),
                14: lambda: outproj_tt(0, [7]),
            })
            c = s_phase(1, 1, c, {
                1: lambda: outproj_tt(0, [8]),
                3: lambda: outproj_tt(0, [9]),
                5: lambda: outproj_tt(0, [10]),
                7: lambda: outproj_tt(0, [11]),
                9: lambda: outproj_tt(0, [12]),
                11: lambda: outproj_tt(0, [13]),
                12: lambda: proj_q_chunk(1, x2["q"], 2),
                13: lambda: outproj_tt(0, [14]),
                14: lambda: outproj_tt(0, [15]),
            })
            c = s_phase(1, 2, c, {
                1: lambda: outproj_tt(1, [0]),
                4: lambda: outproj_tt(1, [1]),
                7: lambda: outproj_tt(1, [2]),
                10: lambda: outproj_tt(1, [3]),
                12: lambda: proj_q_chunk(1, x2["q"], 3),
            })
            c = s_phase(1, 3, c, {
                1: lambda: outproj_tt(1, [4]),
                4: lambda: outproj_tt(1, [5]),
                7: lambda: outproj_tt(1, [6]),
                10: lambda: outproj_tt(1, [7]),
            })
            # drain: PV of the last chunk interleaved with ready outprojs
            # (oc copies alternate scalar/vector - ACT is free by now)
            for kb in range(NKB):
                pv_pair(c, kb, kb == NKB - 1)
                if kb in (2, 6, 10, 14):
                    outproj_tt(1, [8 + kb // 4], on_scalar=(kb % 8 == 2))
            cd_finish(c)
            outproj_tt(1, [12], on_scalar=True)
            outproj_tt(1, [13])
            outproj_tt(1, [14], on_scalar=True)
            outproj_tt(1, [15])

    nc.compile()
    return nc


def _get_prog(masked: bool):
    key = masked
    if key not in _prog_cache:
        _prog_cache[key] = _build(masked)
    return _prog_cache[key]


def kernel(query, key, value, mask, Wq, bq, Wk, bk, Wv, bv, Wo, bo):
    from concourse.bass_utils import run_bass_kernel_spmd

    query = np.asarray(query)
    key = np.asarray(key)
    value = np.asarray(value)
    mask = np.asarray(mask)
    Wq, bq = np.asarray(Wq), np.asarray(bq)
    Wk, bk = np.asarray(Wk), np.asarray(bk)
    Wv, bv = np.asarray(Wv), np.asarray(bv)
    Wo, bo = np.asarray(Wo), np.asarray(bo)

    masked = not bool(mask.all())
    nc = _get_prog(masked)

    def t16(x):  # [S, B, D] -> contiguous [D, B, S] fp16
        return np.ascontiguousarray(x.transpose(2, 1, 0).astype(np.float16))

    def warr(W, hs):  # [128, KT*128]: row p = concat_kt W[hs+m, kt*128+p]
        wt = W[hs:hs + DLOC, :].T.astype(np.float16)       # [kt*128+p, m]
        return np.ascontiguousarray(
            wt.reshape(KT, 128, DLOC).transpose(1, 0, 2).reshape(128, KT * DLOC))

    xq, xk, xv = t16(query), t16(key), t16(value)
    mb = np.where(mask.reshape(S), 0.0, -1e30).astype(np.float32)

    in_maps = []
    for c in range(NCORES):
        hs = c * DLOC
        in_maps.append({
            "xq": xq, "xk": xk, "xv": xv,
            "wq": warr(Wq, hs),
            "wk": warr(Wk, hs),
            "wv": warr(Wv, hs),
            "wo": np.ascontiguousarray(Wo[:, hs:hs + DLOC].T.astype(np.float16)),
            "bq": bq[hs:hs + DLOC].astype(np.float32),
            "bk": bk[hs:hs + DLOC].astype(np.float32),
            "bv": bv[hs:hs + DLOC].astype(np.float32),
            "mb": mb,
        })

    res = run_bass_kernel_spmd(nc, in_maps, core_ids=list(range(NCORES)))
    acc = res.results[0]["out"].astype(np.float64)
    for c in range(1, NCORES):
        acc += res.results[c]["out"]
    acc += bo.astype(np.float64)
    return acc.astype(np.float32)


# revision 32
# speedup vs baseline: 1.0777x; 1.0386x over previous
"""Multi-head attention (S=2048, B=2, D=1024, H=16) on 8 Trainium2 NeuronCores.

Sharding: tensor-parallel over heads. Each core computes 2 heads end-to-end
(QKV projections restricted to its 128 output dims, attention, and the
row-parallel slice of the output projection). The host sums the 8 partial
outputs (row-parallel Wo ==> partial sums) and adds bo.

All compute is fp16 with fp32 PSUM accumulation (fp8 was measured to cost
~8% output error: attention concentrates on few keys, so quantization noise
does not average out). The softmax denominator comes free from a ones-column
appended to V.

Design notes from profiling:
- Full-row (128-contraction) matmuls hide LDWEIGHTS in the background
  weight buffer (216ns steady at N=512); tile_position'd 64-row matmuls do
  not. So per-head K is stored zero-padded to 128 rows and score matmuls
  are plain full-row ones.
- The PE FIFO means any instruction waiting on a semaphore blocks all later
  PE work. The pipeline is software-pipelined one q-chunk deep: during the
  score/exp phase of chunk i, the PE interleaves the PV accumulation of
  chunk i-1 (whose exp tiles are all ready), plus projection/output fillers.
- Input DMAs are 1MB column-quarters of one big [128, KT, S] tile per
  tensor (small DMAs pay a ~1us fixed cost each and starve the startup).
"""

import math

import numpy as np

S, B, D, H = 2048, 2, 1024, 16
DK = D // H               # 64
NCORES = 8
HLOC = H // NCORES        # heads per core = 2
DLOC = HLOC * DK          # local output dims per core = 128
T = S * B                 # tokens = 4096
KT = D // 128             # contraction tiles = 8
NQC = S // 512            # query chunks per batch = 4
NKB = S // 128            # key blocks = 16
NTT = S // 128            # token tiles per batch = 16
SCALE = 1.0 / math.sqrt(DK)

_prog_cache = {}


def _build(masked: bool):
    import concourse.mybir as mybir
    import concourse.tile as tile
    from concourse import bacc

    f16 = mybir.dt.float16
    f32 = mybir.dt.float32
    EXP = mybir.ActivationFunctionType.Exp
    MUL = mybir.AluOpType.mult
    ADD = mybir.AluOpType.add

    nc = bacc.Bacc("TRN2", target_bir_lowering=False, debug=False)

    def din(name, shape, dt=f16):
        return nc.dram_tensor(name, shape, dt, kind="ExternalInput").ap()

    xq = din("xq", [D, B, S])          # query^T
    xk = din("xk", [D, B, S])          # key^T
    xv = din("xv", [D, B, S])          # value^T
    wq = din("wq", [128, KT * DLOC])   # [p, kt, m] = W[hs+m, kt*128+p]
    wk = din("wk", [128, KT * DLOC])
    wv = din("wv", [128, KT * DLOC])
    wo = din("wo", [DLOC, D])          # Wo[:, hs:hs+128].T
    bq = din("bq", [DLOC], f32)
    bk = din("bk", [DLOC], f32)
    bv = din("bv", [DLOC], f32)
    mb = din("mb", [S], f32)           # additive mask bias per key (0 / -1e30)
    out = nc.dram_tensor("out", [S, B, D], f16, kind="ExternalOutput").ap()

    with tile.TileContext(nc) as tc:
        with (
            tc.tile_pool(name="wsb", bufs=1) as wsb,
            tc.tile_pool(name="xsb", bufs=1) as xsb,
            tc.tile_pool(name="qkv", bufs=1) as qkv,
            tc.tile_pool(name="esb", bufs=18) as esb,
            tc.tile_pool(name="nrm", bufs=2) as nrm,
            tc.tile_pool(name="osb", bufs=4) as osb,
            tc.tile_pool(name="pj", bufs=2, space="PSUM") as pj,
            tc.tile_pool(name="psc", bufs=2, space="PSUM") as psc,
            tc.tile_pool(name="pcx", bufs=1, space="PSUM") as pcx,
        ):
            # ---- constants / weights -------------------------------------
            w_sb = {}
            for name, ap in (("wq", wq), ("wk", wk), ("wv", wv)):
                t = wsb.tile([128, KT, DLOC], f16, tag=name)
                nc.sync.dma_start(out=t, in_=ap.rearrange("p (kt m) -> p kt m", kt=KT))
                w_sb[name] = t
            wo_sb = wsb.tile([DLOC, D], f16, tag="wo")
            nc.sync.dma_start(out=wo_sb, in_=wo)
            bq_sb = wsb.tile([DLOC, 1], f32, tag="bq")
            nc.sync.dma_start(out=bq_sb, in_=bq.unsqueeze(1))
            bk_sb = wsb.tile([DLOC, 1], f32, tag="bk")
            nc.sync.dma_start(out=bk_sb, in_=bk.unsqueeze(1))
            bv_row = wsb.tile([1, DLOC], f32, tag="bv_row")
            nc.sync.dma_start(out=bv_row, in_=bv.unsqueeze(0))
            bv_bc = wsb.tile([128, DLOC], f32, tag="bv_bc")
            nc.gpsimd.partition_broadcast(bv_bc, bv_row)
            mb_sb = wsb.tile([128, NKB], f32, tag="mb")
            nc.sync.dma_start(out=mb_sb, in_=mb.rearrange("(kb p) -> p kb", p=128))
            # preload the exp table set while input DMAs are in flight
            warm_e = wsb.tile([1, 1], f16, tag="warm_e")
            nc.scalar.activation(warm_e, mb_sb[0:1, 0:1], EXP, scale=1.0)

            # persistent per-batch activations
            qT = [qkv.tile([DLOC, S], f16, tag=f"qT{b}", name=f"qT{b}") for b in range(B)]
            # per-head K, zero-padded to 128 rows: head h occupies rows
            # h*64:(h+1)*64, the rest stays 0 -> score matmuls are full-row
            # (LDWEIGHTS hides in the background weight buffer).
            kTz = [[qkv.tile([128, S], f16, tag=f"kT{b}{h}", name=f"kT{b}{h}")
                    for h in range(HLOC)] for b in range(B)]
            # V per (key-block, head): [keys=128, 68] with ones in col 64
            vv = [qkv.tile([128, NKB, HLOC, 68], f16, tag=f"vv{b}", name=f"vv{b}")
                  for b in range(B)]
            for b in range(B):
                for h in range(HLOC):
                    nc.vector.memset(kTz[b][h], 0.0)
                nc.vector.memset(vv[b], 0.0)
                nc.vector.memset(vv[b][:, :, :, 64:65], 1.0)
            ctxn = [qkv.tile([DLOC, S], f16, tag=f"ctxn{b}", name=f"ctxn{b}") for b in range(B)]

            def load_x(ap, name, b, eng):
                """One big [128, KT, S] tile, four 1MB column-quarter DMAs
                (all kt per quarter, so proj chunk c unblocks at quarter c).
                Returns per-kt view APs."""
                big = xsb.tile([128, KT, S], f16, tag=f"x{name}", name=f"x{name}{b}")
                src = ap[:, b, :].rearrange("(kt p) s -> p kt s", p=128)
                for i in range(4):
                    eng.dma_start(out=big[:, :, i * 512:(i + 1) * 512],
                                  in_=src[:, :, i * 512:(i + 1) * 512])
                return [big[:, kt, :] for kt in range(KT)]

            def proj_q_chunk(b, xt, qc):
                ps = pj.tile([DLOC, 512], f32, tag="pj", name="ps")
                sl = slice(qc * 512, (qc + 1) * 512)
                for kt in range(KT):
                    nc.tensor.matmul(ps, w_sb["wq"][:, kt, :], xt[kt][:, sl],
                                     start=(kt == 0), stop=(kt == KT - 1))
                nc.vector.tensor_scalar(out=qT[b][:, sl], in0=ps,
                                        scalar1=bq_sb, scalar2=None, op0=ADD)

            def proj_k_chunk(b, xt, qc):
                ps = pj.tile([DLOC, 512], f32, tag="pj", name="ps")
                sl = slice(qc * 512, (qc + 1) * 512)
                for kt in range(KT):
                    nc.tensor.matmul(ps, w_sb["wk"][:, kt, :], xt[kt][:, sl],
                                     start=(kt == 0), stop=(kt == KT - 1))
                # split the two heads into their zero-padded K tiles
                for h in range(HLOC):
                    hsl = slice(h * 64, (h + 1) * 64)
                    nc.vector.tensor_scalar(out=kTz[b][h][hsl, sl],
                                            in0=ps[hsl, :],
                                            scalar1=bk_sb[hsl, :], scalar2=None,
                                            op0=ADD)

            def proj_v_tt(b, xt, tts):
                for tt in tts:
                    ps = pj.tile([128, DLOC], f32, tag="pj", name="ps")
                    sl = slice(tt * 128, (tt + 1) * 128)
                    for kt in range(KT):
                        nc.tensor.matmul(ps, xt[kt][:, sl], w_sb["wv"][:, kt, :],
                                         start=(kt == 0), stop=(kt == KT - 1))
                    for h in range(HLOC):
                        nc.vector.tensor_tensor(
                            out=vv[b][:, tt, h, 0:64],
                            in0=ps[:, h * 64:(h + 1) * 64],
                            in1=bv_bc[:, h * 64:(h + 1) * 64], op=ADD)

            def outproj_tt(b, tts, on_scalar=False, store_eng=None):
                store = store_eng or nc.gpsimd
                for tt in tts:
                    tsl = slice(tt * 128, (tt + 1) * 128)
                    for eh in range(2):
                        po = pj.tile([128, 512], f32, tag="pj", name="po")
                        nc.tensor.matmul(po, ctxn[b][:, tsl],
                                         wo_sb[:, eh * 512:(eh + 1) * 512],
                                         start=True, stop=True)
                        oc = osb.tile([128, 512], f16, tag="oc", name="oc")
                        if on_scalar:
                            nc.scalar.copy(oc, po)
                        else:
                            nc.vector.tensor_copy(oc, po)
                        store.dma_start(
                            out=out[tsl, b, eh * 512:(eh + 1) * 512], in_=oc)

            # ---- software-pipelined attention ----------------------------
            # carry = (b, qc, ets, pctx) of the previous q-chunk whose PV
            # accumulation runs interleaved into the current s-phase.
            def pv_pair(carry, kb, last):
                cb, cqc, ets, pctx = carry
                et = ets.pop(kb)
                for h in range(HLOC):
                    nc.tensor.matmul(pctx[h], vv[cb][:, kb, h, 0:65],
                                     et[:, h, :],
                                     start=(kb == 0), stop=(kb == NKB - 1))

            def cd_finish(carry):
                cb, cqc, ets, pctx = carry
                qsl = slice(cqc * 512, (cqc + 1) * 512)
                for h in range(HLOC):
                    hsl = slice(h * 64, (h + 1) * 64)
                    cl = nrm.tile([1, 512], f32, tag="cl", name="cl")
                    nc.vector.tensor_copy(cl, pctx[h][64:65, :])
                    rl = nrm.tile([1, 512], f32, tag="rl", name="rl")
                    nc.vector.reciprocal_approx_fast(rl, cl)
                    rl_bc = nrm.tile([64, 512], f32, tag="rlb", name="rlb")
                    nc.gpsimd.partition_broadcast(rl_bc, rl)
                    nc.vector.tensor_tensor(out=ctxn[cb][hsl, qsl],
                                            in0=pctx[h][0:64, :], in1=rl_bc,
                                            op=MUL)

            def s_phase(b, qc, carry, inj):
                """16 score+exp steps; interleaves carry's PV pairs and the
                injected fillers. Returns this chunk's carry."""
                qsl = slice(qc * 512, (qc + 1) * 512)
                pctx = [pcx.tile([65, 512], f32, tag=f"cx{h}", name=f"cx{h}")
                        for h in range(HLOC)]
                ets = {}
                LAG = 2   # carry-PV trails by 2 steps so same-step injected
                # producers (e.g. vproj feeding vv) are emitted in time
                for kb in range(NKB):
                    t = psc.tile([128, HLOC, 512], f32, tag="sc", name="sc")
                    ksl = slice(kb * 128, (kb + 1) * 128)
                    for h in range(HLOC):
                        nc.tensor.matmul(t[:, h, :], kTz[b][h][:, ksl],
                                         qT[b][:, qsl], start=True, stop=True)
                    et = esb.tile([128, HLOC, 512], f16, tag="e", name="et")
                    if masked:
                        nc.scalar.activation(et, t, EXP,
                                             bias=mb_sb[:, kb:kb + 1], scale=SCALE)
                    else:
                        nc.scalar.activation(et, t, EXP, scale=SCALE)
                    ets[kb] = et
                    if carry is not None and kb >= LAG:
                        pv_pair(carry, kb - LAG, False)
                    th = inj.get(kb)
                    if th is not None:
                        th()
                if carry is not None:
                    for kb in range(NKB - LAG, NKB):
                        pv_pair(carry, kb, kb == NKB - 1)
                    cd_finish(carry)
                return (b, qc, ets, pctx)

            # ---- schedule ------------------------------------------------
            # DMA queues: sync = xk0, xk1, xq1; scalar = xq0 then xv0 (the
            # FIFO delays xv0 until xq0 is fully landed, and xv0's four
            # issue slots are the only scalar DMAs inside the exp stream);
            # gpsimd = xv1 (its tile-ring WAR delays it behind vproj b0)
            # and the output stores.
            xk_t = load_x(xk, "k", 0, nc.sync)
            xq_t = load_x(xq, "q", 0, nc.scalar)
            for wu in range(40):
                jp = pj.tile([128, 512], f32, tag="pj", name="jp")
                nc.tensor.matmul(jp, w_sb["wq"][:, wu % 8, :],
                                 w_sb["wk"][:, (wu % 2) * 4:(wu % 2) * 4 + 4, :],
                                 start=True, stop=True)
            proj_k_chunk(0, xk_t, 0)
            proj_q_chunk(0, xq_t, 0)
            # gate xv0's transfers behind the first k-proj PSUM->SBUF copy so
            # it can't steal HBM bandwidth from the xk0/xq0 ramp: a 1-element
            # copy into the xv tile makes the DMAs' WAR wait on it
            xv_big = xsb.tile([128, KT, S], f16, tag="xv", name="xv0")
            nc.vector.tensor_copy(xv_big[0:1, 0:1, 0:1], qT[0][0:1, 0:1])
            xv_src = xv[:, 0, :].rearrange("(kt p) s -> p kt s", p=128)
            for i in range(4):
                nc.gpsimd.dma_start(out=xv_big[:, :, i * 512:(i + 1) * 512],
                                    in_=xv_src[:, :, i * 512:(i + 1) * 512])
            xv_t = [xv_big[:, kt, :] for kt in range(KT)]
            x2 = {"v": xv_t}

            # Ring hazard rule: a tensor's generation-2 load must be emitted
            # AFTER the last emitted reader of generation 1 (bufs=1 rings).
            c = s_phase(0, 0, None, {
                1: lambda: proj_k_chunk(0, xk_t, 1),
                4: lambda: proj_k_chunk(0, xk_t, 2),
                7: lambda: proj_k_chunk(0, xk_t, 3),
                8: lambda: x2.update(k=load_x(xk, "k", 1, nc.sync)),
                9: lambda: proj_q_chunk(0, xq_t, 1),
            })
            c = s_phase(0, 1, c, {
                1: lambda: proj_v_tt(0, x2["v"], [0, 1]),
                2: lambda: proj_v_tt(0, x2["v"], [2, 3]),
                3: lambda: proj_v_tt(0, x2["v"], [4, 5]),
                4: lambda: proj_v_tt(0, x2["v"], [6, 7]),
                5: lambda: proj_v_tt(0, x2["v"], [8, 9]),
                6: lambda: proj_v_tt(0, x2["v"], [10, 11]),
                7: lambda: proj_v_tt(0, x2["v"], [12, 13]),
                8: lambda: proj_v_tt(0, x2["v"], [14, 15]),
                10: lambda: x2.update(v2=load_x(xv, "v", 1, nc.gpsimd)),
                14: lambda: proj_q_chunk(0, xq_t, 2),
            })
            c = s_phase(0, 2, c, {
                1: lambda: proj_q_chunk(0, xq_t, 3),
                2: lambda: x2.update(q=load_x(xq, "q", 1, nc.sync)),
                3: lambda: proj_k_chunk(1, x2["k"], 0),
                6: lambda: proj_k_chunk(1, x2["k"], 1),
                9: lambda: proj_k_chunk(1, x2["k"], 2),
                12: lambda: proj_k_chunk(1, x2["k"], 3),
            })
            c = s_phase(0, 3, c, {
                1: lambda: proj_q_chunk(1, x2["q"], 0),
                4: lambda: proj_q_chunk(1, x2["q"], 1),
                7: lambda: outproj_tt(0, [0]),
                8: lambda: outproj_tt(0, [1]),
                9: lambda: outproj_tt(0, [2]),
                10: lambda: outproj_tt(0, [3]),
                11: lambda: outproj_tt(0, [4]),
                12: lambda: outproj_tt(0, [5]),
                13: lambda: proj_v_tt(1, x2["v2"], [0, 1]),
                14: lambda: proj_v_tt(1, x2["v2"], [2, 3]),
            })
            c = s_phase(1, 0, c, {
                1: lambda: proj_v_tt(1, x2["v2"], [4, 5]),
                3: lambda: proj_v_tt(1, x2["v2"], [6, 7]),
                5: lambda: proj_v_tt(1, x2["v2"], [8, 9]),
                7: lambda: proj_v_tt(1, x2["v2"], [10, 11]),
                9: lambda: proj_v_tt(1, x2["v2"], [12, 13]),
                11: lambda: proj_v_tt(1, x2["v2"], [14, 15]),
                13: lambda: outproj_tt(0, [6]),
                14: lambda: outproj_tt(0, [7]),
            })
            c = s_phase(1, 1, c, {
                1: lambda: outproj_tt(0, [8]),
                3: lambda: outproj_tt(0, [9]),
                5: lambda: outproj_tt(0, [10]),
                7: lambda: outproj_tt(0, [11]),
                9: lambda: outproj_tt(0, [12]),
                11: lambda: outproj_tt(0, [13]),
                12: lambda: proj_q_chunk(1, x2["q"], 2),
                13: lambda: outproj_tt(0, [14]),
                14: lambda: outproj_tt(0, [15]),
            })
            c = s_phase(1, 2, c, {
                1: lambda: outproj_tt(1, [0]),
                4: lambda: outproj_tt(1, [1]),
                7: lambda: outproj_tt(1, [2]),
                10: lambda: outproj_tt(1, [3]),
                12: lambda: proj_q_chunk(1, x2["q"], 3),
            })
            c = s_phase(1, 3, c, {
                1: lambda: outproj_tt(1, [4]),
                4: lambda: outproj_tt(1, [5]),
                7: lambda: outproj_tt(1, [6]),
                10: lambda: outproj_tt(1, [7]),
            })
            # drain: PV of the last chunk interleaved with ready outprojs
            # (oc copies alternate scalar/vector - ACT is free by now; stores
            # ride the idle sync HWDGE queue for a faster end barrier)
            for kb in range(NKB):
                pv_pair(c, kb, kb == NKB - 1)
                if kb in (2, 6, 10, 14):
                    outproj_tt(1, [8 + kb // 4], on_scalar=(kb % 8 == 2),
                               store_eng=nc.sync)
            cd_finish(c)
            outproj_tt(1, [12], on_scalar=True, store_eng=nc.sync)
            outproj_tt(1, [13], store_eng=nc.sync)
            outproj_tt(1, [14], on_scalar=True, store_eng=nc.sync)
            outproj_tt(1, [15], store_eng=nc.sync)

    nc.compile()
    return nc


def _get_prog(masked: bool):
    key = masked
    if key not in _prog_cache:
        _prog_cache[key] = _build(masked)
    return _prog_cache[key]


def kernel(query, key, value, mask, Wq, bq, Wk, bk, Wv, bv, Wo, bo):
    from concourse.bass_utils import run_bass_kernel_spmd

    query = np.asarray(query)
    key = np.asarray(key)
    value = np.asarray(value)
    mask = np.asarray(mask)
    Wq, bq = np.asarray(Wq), np.asarray(bq)
    Wk, bk = np.asarray(Wk), np.asarray(bk)
    Wv, bv = np.asarray(Wv), np.asarray(bv)
    Wo, bo = np.asarray(Wo), np.asarray(bo)

    masked = not bool(mask.all())
    nc = _get_prog(masked)

    def t16(x):  # [S, B, D] -> contiguous [D, B, S] fp16
        return np.ascontiguousarray(x.transpose(2, 1, 0).astype(np.float16))

    def warr(W, hs):  # [128, KT*128]: row p = concat_kt W[hs+m, kt*128+p]
        wt = W[hs:hs + DLOC, :].T.astype(np.float16)       # [kt*128+p, m]
        return np.ascontiguousarray(
            wt.reshape(KT, 128, DLOC).transpose(1, 0, 2).reshape(128, KT * DLOC))

    xq, xk, xv = t16(query), t16(key), t16(value)
    mb = np.where(mask.reshape(S), 0.0, -1e30).astype(np.float32)

    in_maps = []
    for c in range(NCORES):
        hs = c * DLOC
        in_maps.append({
            "xq": xq, "xk": xk, "xv": xv,
            "wq": warr(Wq, hs),
            "wk": warr(Wk, hs),
            "wv": warr(Wv, hs),
            "wo": np.ascontiguousarray(Wo[:, hs:hs + DLOC].T.astype(np.float16)),
            "bq": bq[hs:hs + DLOC].astype(np.float32),
            "bk": bk[hs:hs + DLOC].astype(np.float32),
            "bv": bv[hs:hs + DLOC].astype(np.float32),
            "mb": mb,
        })

    res = run_bass_kernel_spmd(nc, in_maps, core_ids=list(range(NCORES)))
    acc = res.results[0]["out"].astype(np.float64)
    for c in range(1, NCORES):
        acc += res.results[c]["out"]
    acc += bo.astype(np.float64)
    return acc.astype(np.float32)
